# revision 17
# baseline (speedup 1.0000x reference)
"""Distributed Trainium2 kernel for BCESleepLoss.

loss = mean(weight_c * (softplus(x) - x*t)) + 1e-4 * sum_n sum_j corr_n[j]^2 / norm_n

where corr_n = full cross-correlation of predictions[n,:,1] with predictions[n,:,2]
and norm_n = sqrt(sum(s1^2) * sum(s2^2)).

Sharding: data-parallel over the batch dim N=32 -> 4 samples on each of 8 cores.
Each core emits per-partition partial stats [128, 16]; the host does the final
(tiny) reduction in float64.

Cross-correlation as matmuls: for each sample, with K=128,
  out[m', nu] += A_cols[:, i:i+128].T @ B_sh[:, 128*i : 128*i+128],  i = 0..64
where A_cols[tau, g] = a_pad[128*g + tau] (zero-padded reshape of s1, built
on-chip via PE transposes) and B_sh[tau, x] = b_pad[tau + x + 1] (128 shifted
copies of zero-padded s2, staged through a DRAM scratch so a single
overlapping-read DMA can build it).  The 128x128 PSUM tile then holds every
correlation lag exactly once (scrambled), so sum(out^2) == sum(corr^2).
Verified against np.convolve in float64.

v3 schedule: the critical chain to the first matmul is
  x_sb(partitions 0:32) -> b_de cast -> b_pad write(s0) -> B_sh chunk read
with each link as small as possible (split input DMA, sample-0-first staging,
a small 1024-col first chunk) because every DMA link costs ~2-3.5us of fixed
dispatch+completion latency.  A_cols for all 4 samples are built in 6 batched
DVE ops.  BCE runs on Scalar/DVE idle time during the matmul stream and uses
the direct softplus form ln(1+e^x) - x*t (equal to the reference's stable
form; |x| < ~6 here so e^x cannot overflow).
"""

import numpy as np

import concourse.bass as bass
import concourse.mybir as mybir
import concourse.tile as tile
from concourse import bacc
from concourse.bass_utils import run_bass_kernel_spmd
from concourse.masks import make_identity

# Problem constants (hardcoded; kernel.py must be self-contained).
N_FULL = 32
L = 8192
C = 3
LAMBDA1 = 1.0
LAMBDA2 = 1e-4

N_CORES = 8
NS = N_FULL // N_CORES  # samples per core = 4

K = 128  # partition / tile size
G = L // K  # 64 columns of signal data per sample
NT = G + 1  # 65 accumulating matmuls per sample
A_W = 3 * G  # 192: A_cols width (64 zero | 64 data | 64 zero)
BP_LEN = 8576  # b_pad length = 128*67 (zeros | 8192 data | zeros)
SW = 256  # cols per de-strided signal: NS*L/K
FW = NS * L * C // K  # 768 cols in the flat [128, 768] input layout

# B_sh chunking (128-aligned boundaries).  Sample 0 gets a small first chunk
# so its matmuls can start as early as possible; later samples use 2 chunks.
CHUNKS_S0 = [(0, 2048), (2048, 2048), (4096, 4232)]
CHUNKS_SN = [(0, 4096), (4096, 4232)]

F32 = mybir.dt.float32
BF16 = mybir.dt.bfloat16
FP8 = mybir.dt.float8e4  # e4m3: staging/matmul dtype (rel-err gate is 2e-2)

LAST_RESULT = None  # BassKernelResults of the most recent run (for test.py)
_CACHED_NC = None

FULL_PARTS = ("corr", "bce")


def _kernel_body(tc, parts=FULL_PARTS):
    nc = tc.nc
    pred = nc.dram_tensor("predictions", [NS, L, C], F32, kind="ExternalInput").ap()
    targ = nc.dram_tensor("targets", [NS, L, C], F32, kind="ExternalInput").ap()
    out = nc.dram_tensor("out", [K, 16], F32, kind="ExternalOutput").ap()

    with (
        tc.tile_pool(name="singles", bufs=1) as singles,
        tc.tile_pool(name="bsh", bufs=4) as bsh_pool,
        tc.tile_pool(name="scr", bufs=2) as scr,
        tc.tile_pool(name="bce", bufs=1) as bce_pool,
        tc.tile_pool(name="psum", bufs=2, space="PSUM") as psum_pool,
        tc.tile_pool(name="psumt", bufs=1, space="PSUM") as psumt_pool,
        tc.tile_pool(name="dram", bufs=1, space="DRAM") as dram_pool,
    ):
        # Per-partition partial stats, one DMA out at the end.
        # cols 0:4 = sum(c^2) per sample; col 4 = sum(s1^2), col 5 = sum(s2^2)
        # (per-partition, sample = p // 32); cols 6:9 = per-class BCE sums.
        stats = singles.tile([K, 16], F32)
        nc.vector.memset(stats[:], 0.0)

        pred_flat = pred.rearrange("n l c -> (n l c)").rearrange("(p f) -> p f", p=K)
        x_sb = bce_pool.tile([K, FW], F32)
        x_v = x_sb[:].rearrange("p (t c) -> p c t", c=C)

        if "corr" in parts:
            # Identity for the PE transposes; no input deps, build first.
            ident = singles.tile([K, K], BF16)
            make_identity(nc, ident[:])

            # PE warm-up: the tensor engine boots clock-gated to 1.2GHz and
            # only reaches 2.4GHz after ~3.4us of sustained activity.  Burn
            # dummy accumulating matmuls on the identity during the staging
            # latency window so the real stream runs warm from its first
            # instruction.  Split into two groups so the a_de transposes
            # (which gate the DVE A_cols build chain) slot in between instead
            # of queueing behind the whole warm-up.
            psum_warm = psumt_pool.tile([K, K], F32, tag="warm")
            N_WARM1, N_WARM2 = 28, 40
            for w in range(N_WARM1):
                nc.tensor.matmul(
                    psum_warm[:], ident[:], ident[:],
                    start=(w == 0), stop=(w == N_WARM1 - 1),
                )

            zer = singles.tile([8, 136], FP8)
            nc.gpsimd.memset(zer[:], 0.0)
            b_pad_all = dram_pool.tile([NS * BP_LEN], FP8, name="b_pad_all")
            bpa = b_pad_all[:]

        # Input loads: sample 0's partitions first so its staging chain can
        # start ~0.9us earlier; the rest right behind on the same queue.
        nc.sync.dma_start(out=x_sb[0:32, :], in_=pred_flat[0:32, :])
        nc.sync.dma_start(out=x_sb[32:K, :], in_=pred_flat[32:K, :])

        if "corr" in parts:
            # b_pad zero gaps: chunk reads touch bytes [1,128) and
            # [8320,8456) of each sample's region only, so zero just those.
            # On the sync queue so the chunk reads behind them need no
            # cross-queue semaphore wait (same-ring FIFO ordering).
            nc.sync.dma_start(
                out=bass.AP(
                    tensor=bpa.tensor, offset=bpa.offset,
                    ap=[[BP_LEN, NS], [1, K]],
                ),
                in_=zer[0:4, 0:K],
            )
            nc.sync.dma_start(
                out=bass.AP(
                    tensor=bpa.tensor, offset=bpa.offset + 8320,
                    ap=[[BP_LEN, NS], [1, 136]],
                ),
                in_=zer[0:4, 0:136],
            )

            # De-stride s2 + cast to fp8 (DVE): b_de[p, u] = s2[p//32][256*(p%32)+u]
            b_de = singles.tile([K, SW], FP8)
            nc.vector.tensor_copy(out=b_de[0:32, :], in_=x_v[0:32, 2, :])
            # b_pad data write for sample 0 alone: gates the first chunk read.
            nc.sync.dma_start(
                out=bass.AP(
                    tensor=bpa.tensor, offset=bpa.offset + K,
                    ap=[[SW, 32], [1, SW]],
                ),
                in_=b_de[0:32, :],
            )
            # (DVE ops must respect 32-aligned partition groups: base 32 can
            # span at most 32 partitions, so split the remainder.)
            nc.vector.tensor_copy(out=b_de[32:64, :], in_=x_v[32:64, 2, :])
            nc.vector.tensor_copy(out=b_de[64:K, :], in_=x_v[64:K, 2, :])

            # B_sh chunk reads, in consumption order: B_sh[tau,x] = b_pad[tau+x+1].
            # Sample 0's chunks A/B ride the sync queue directly behind the
            # zero-fills and the s0 data write (same-ring FIFO, no semaphore
            # round-trips); chunk C transfers in parallel on the scalar
            # queue; samples 1-3 stream on the gpsimd queue.
            def chunk_dma(eng, n, h, off, w):
                b_shc = bsh_pool.tile(
                    [K, w], FP8,
                    tag=f"bsh{'ABC'[h] if n == 0 else h}",
                    name=f"b_sh{n}c{h}",
                )
                qsrc = bass.AP(
                    tensor=bpa.tensor,
                    offset=bpa.offset + n * BP_LEN + 1 + off,
                    ap=[[1, K], [1, w]],
                )
                eng.dma_start(out=b_shc[:], in_=qsrc)
                return (off, w, b_shc)

            # s1-3 data write rides sync right behind write0 so the gpsimd
            # chunk stream unblocks as early as possible.
            nc.sync.dma_start(
                out=bass.AP(
                    tensor=bpa.tensor, offset=bpa.offset + BP_LEN + K,
                    ap=[[BP_LEN, NS - 1], [SW, 32], [1, SW]],
                ),
                in_=b_de[32:K, :],
            )
            b_shs = [[chunk_dma(nc.sync, 0, 0, *CHUNKS_S0[0])]]
            b_shs[0].append(chunk_dma(nc.sync, 0, 1, *CHUNKS_S0[1]))
            # chunk C is emitted below on the scalar queue; s1-3 chunks on
            # gpsimd right here so their serial stream starts the moment the
            # s1-3 data write lands.
            for n in range(1, NS):
                b_shs.append(
                    [chunk_dma(nc.gpsimd, n, h, *s) for h, s in enumerate(CHUNKS_SN)]
                )

            # a-side: de-stride s1 to bf16 (DVE), transpose halves once for
            # ALL samples: a_deT_*[tau, p] = a_de[p, tau (+128)]
            a_de = singles.tile([K, SW], BF16)
            nc.vector.tensor_copy(out=a_de[:], in_=x_v[:, 1, :])
            a_te = psumt_pool.tile([K, K], BF16, tag="a_te")
            nc.tensor.transpose(a_te[:], a_de[:, 0:K], ident[:])
            a_to = psumt_pool.tile([K, K], BF16, tag="a_to")
            nc.tensor.transpose(a_to[:], a_de[:, K : 2 * K], ident[:])
            # warm-up, part 2: keep the PE clock released until the first
            # B_sh chunk semaphore fires.
            psum_warm2 = psumt_pool.tile([K, K], F32, tag="warm")
            for w in range(N_WARM2):
                nc.tensor.matmul(
                    psum_warm2[:], ident[:], ident[:],
                    start=(w == 0), stop=(w == N_WARM2 - 1),
                )

            # A_cols for all 4 samples in 6 batched DVE ops.  Per sample:
            # [64 zero | a fp8 | 64 zero]; even/odd g columns come from the
            # two transpose halves; 3 column-shifted copies keep every matmul
            # weight slice 4-byte aligned.
            a_cols_all = singles.tile([K, NS * A_W], FP8)
            nc.vector.memset(a_cols_all[:], 0.0)
            acv = a_cols_all[:].rearrange("p (n gt two) -> p n two gt", n=NS, two=2)
            nc.vector.tensor_copy(
                out=acv[:, :, 0, 32:64],
                in_=a_te[:].rearrange("t (n j) -> t n j", n=NS),
            )
            nc.vector.tensor_copy(
                out=acv[:, :, 1, 32:64],
                in_=a_to[:].rearrange("t (n j) -> t n j", n=NS),
            )
            a_phs = [a_cols_all]
            for r in range(1, 4):
                a_ph = singles.tile([K, NS * A_W], FP8, name=f"a_ph{r}")
                nc.vector.tensor_copy(
                    out=a_ph[:].rearrange("p (n f) -> p n f", n=NS)[:, :, 0 : A_W - r],
                    in_=a_cols_all[:].rearrange("p (n f) -> p n f", n=NS)[:, :, r:A_W],
                )
                a_phs.append(a_ph)

            # Scalar queue: BCE exp first (ready as soon as x lands), then
            # sample 0's chunk C (transfers in parallel with A/B on sync);
            # ln and the t_sb load follow below.
            if "bce" in parts:
                ex = bce_pool.tile([K, FW], F32)
                nc.scalar.activation(ex[:], x_sb[:], mybir.ActivationFunctionType.Exp)
            b_shs[0].append(chunk_dma(nc.scalar, 0, 2, *CHUNKS_S0[2]))

            # 65 accumulating matmuls per sample; psum holds every corr lag
            # exactly once.
            psums = []
            for n in range(NS):
                chunks = b_shs[n]
                psum = psum_pool.tile([K, K], F32)
                for i in range(NT):
                    r = i % 4
                    w0 = n * A_W + i - r
                    off, w, b_shc = next(
                        c for c in reversed(chunks) if c[0] <= K * i
                    )
                    c0 = K * i - off
                    nc.tensor.matmul(
                        psum[:],
                        a_phs[r][:, w0 : w0 + K],
                        b_shc[:, c0 : c0 + K],
                        start=(i == 0),
                        stop=(i == NT - 1),
                    )
                psums.append(psum)

        if "bce" in parts:
            # ---- BCE (cont.): ln(1 + exp(x)) - x*t, per-class sums.  Runs
            # on Scalar/DVE idle time during the matmul stream. ----
            if "corr" not in parts:
                ex = bce_pool.tile([K, FW], F32)
                nc.scalar.activation(ex[:], x_sb[:], mybir.ActivationFunctionType.Exp)
            sp = bce_pool.tile([K, FW], F32)
            nc.scalar.activation(
                sp[:], ex[:], mybir.ActivationFunctionType.Ln, bias=1.0
            )
            # t_sb on the scalar queue, after the critical-path DMAs, so its
            # HBM transfer never competes with the staging chain.
            t_sb = bce_pool.tile([K, FW], F32)
            nc.scalar.dma_start(
                out=t_sb[:],
                in_=targ.rearrange("n l c -> (n l c)").rearrange(
                    "(p f) -> p f", p=K
                ),
            )
            xt = bce_pool.tile([K, FW], F32)
            nc.vector.tensor_mul(xt[:], x_sb[:], t_sb[:])
            v = bce_pool.tile([K, FW], F32)
            nc.vector.tensor_sub(v[:], sp[:], xt[:])
            v_view = v[:].rearrange("p (t c) -> p c t", c=C)
            nc.vector.reduce_sum(
                stats[:, 6 : 6 + C], v_view, axis=mybir.AxisListType.X
            )

        if "corr" in parts:
            # norms in f32 from x_sb: per-partition partials (sample = p//32)
            scr_n = scr.tile([K, SW], F32, tag="scr_n")
            nc.vector.tensor_mul(scr_n[:], x_v[:, 1, :], x_v[:, 1, :])
            nc.vector.reduce_sum(stats[:, 4:5], scr_n[:], axis=mybir.AxisListType.X)
            scr_n2 = scr.tile([K, SW], F32, tag="scr_n")
            nc.vector.tensor_mul(scr_n2[:], x_v[:, 2, :], x_v[:, 2, :])
            nc.vector.reduce_sum(stats[:, 5:6], scr_n2[:], axis=mybir.AxisListType.X)

            # sum(c^2) -> stats col n (square on ScalarE, reduce on DVE);
            # emitted after the BCE scalar ops so Square(s0) frees psum bank 0
            # well before sample 2's matmuls need it.
            for n in range(NS):
                scr_c2 = scr.tile([K, K], F32, tag="scr_c2")
                nc.scalar.activation(
                    out=scr_c2[:], in_=psums[n][:],
                    func=mybir.ActivationFunctionType.Square,
                )
                nc.vector.reduce_sum(
                    stats[:, n : n + 1], scr_c2[:], axis=mybir.AxisListType.X
                )

        nc.sync.dma_start(out=out[:], in_=stats[:])


def _build(parts=FULL_PARTS):
    global _CACHED_NC
    if _CACHED_NC is not None and _CACHED_NC[0] == parts:
        return _CACHED_NC[1]
    nc = bacc.Bacc(
        "TRN2",
        target_bir_lowering=False,
        debug=False,
        enable_asserts=False,
        num_devices=N_CORES,
    )
    with tile.TileContext(nc) as tc:
        _kernel_body(tc, parts)
    nc.compile()
    _CACHED_NC = (parts, nc)
    return nc


def host_reduce(stats_list, weight):
    """Final scalar reduction over per-core [128, 16] stats, in float64."""
    w = np.asarray(weight, dtype=np.float64)
    bce_sum = 0.0
    prox = 0.0
    for stats in stats_list:
        s = np.asarray(stats, dtype=np.float64)
        ss = s[:, 0:4].sum(axis=0)
        sa = s[:, 4].reshape(NS, 32).sum(axis=1)
        sb = s[:, 5].reshape(NS, 32).sum(axis=1)
        prox += float((ss / np.sqrt(sa * sb)).sum())
        bce_sum += float((s[:, 6:9].sum(axis=0) * w).sum())
    loss = LAMBDA1 * bce_sum / (N_FULL * L * C) + LAMBDA2 * prox
    return np.float32(loss)


def kernel(predictions, targets, weight, trace=False):
    global LAST_RESULT
    predictions = np.ascontiguousarray(np.asarray(predictions, dtype=np.float32))
    targets = np.ascontiguousarray(np.asarray(targets, dtype=np.float32))
    weight = np.asarray(weight, dtype=np.float32)
    assert predictions.shape == (N_FULL, L, C), predictions.shape

    nc = _build()
    in_maps = [
        {
            "predictions": np.ascontiguousarray(predictions[k * NS : (k + 1) * NS]),
            "targets": np.ascontiguousarray(targets[k * NS : (k + 1) * NS]),
        }
        for k in range(N_CORES)
    ]
    LAST_RESULT = run_bass_kernel_spmd(
        nc, in_maps, core_ids=list(range(N_CORES)), trace=trace
    )
    stats_list = [r["out"] for r in LAST_RESULT.results]
    return host_reduce(stats_list, weight)
